# revision 30
# baseline (speedup 1.0000x reference)
"""Multi-head GAT layer (4 heads, mean-aggregated) + residual + GraphNorm + gelu
on 8 Trainium2 NeuronCores (SPMD, one NEFF on all cores).

v3 strategy:
  - dst nodes are dealt to (core, tile, slot) round-robin by in-degree so per
    tile edge counts are balanced across cores (less SPMD max-padding).
  - NO on-device gather (SWDGE descriptor generation is serial on the Pool
    engine at ~2-8ns/descriptor -> ~1ms floor for per-edge gathers).  Instead
    the host lays out the per-edge source rows x[src_e] directly in transposed
    block-aligned order (xedgeT[f, edge_slot]) and the kernel streams them as
    bulk contiguous DMA.  Edge slots are grouped as 98 dst tiles x ~13 blocks
    of 128 edges.
  - Per 128-edge block, PE recomputes xl = x@W.T (and a_src = x@As.T); the
    one-hot-distributed a_dst (ape) accumulates onto a_src in one PSUM tile
    per dst tile, so leakyrelu+exp run batched per tile on ACT.
  - One-hot matrices (m1: [e,d], m2t: [d,e]) are uploaded as fp8 (exact 0/1)
    and used directly as matmul weights against bf16 moving operands.
  - The per-edge coefficient multiply (rhs = xl * ex) runs on two-block PSUM
    tiles and is split between DVE and ACT.
  - Softmax denominators ride as 4 extra columns of the agg matmul rhs.
  - GraphNorm stats accumulate in a persistent PSUM bank across all tiles,
    then AllReduce across the 8 cores.
"""
import numpy as np
import ml_dtypes

_BF16 = ml_dtypes.bfloat16
_FP8 = ml_dtypes.float8_e4m3
_F32 = np.float32

N, F, C, H, E, B = 100000, 128, 64, 4, 1200000, 8
NCORE = 8
NEG = 0.2
EPS = 1e-5
NOWN = N // NCORE             # 12500 dst nodes per core
TILES = (NOWN + 127) // 128   # 98 dst tiles per core (last has 84 slots)
LAST_ROWS = NOWN - (TILES - 1) * 128
G = 7                         # dst tiles per DMA group (98 = 14*7)
DVE_MOD = 8                   # duos with (idx % DVE_MOD) < DVE_CUT multiply on DVE
DVE_CUT = 7


def _cdiv(a, b):
    return (a + b - 1) // b


def _assign_nodes(deg):
    """Deal nodes (by descending degree) round-robin over the 784 (core,tile)
    pairs, honoring per-tile capacity. Returns [N] arrays core, tile, slot."""
    P = NCORE * TILES
    caps = np.full(P, 128, np.int64)
    caps[TILES - 1::TILES] = LAST_ROWS
    order = np.argsort(-deg, kind="stable")
    fill = np.zeros(P, np.int64)
    core = np.empty(N, np.int64)
    tile = np.empty(N, np.int64)
    slot = np.empty(N, np.int64)
    p = 0
    for n in order:
        while fill[p] >= caps[p]:
            p = (p + 1) % P
        core[n] = p // TILES
        tile[n] = p % TILES
        slot[n] = fill[p]
        fill[p] += 1
        p = (p + 1) % P
    return core, tile, slot


def _host_prep(x, edge_index, batch, W, att_src, att_dst, bias_gat, res_W,
               res_b, gn_weight, gn_bias, gn_mean_scale):
    x = np.asarray(x, _F32)
    W = np.asarray(W, _F32)
    att_src = np.asarray(att_src, _F32)
    att_dst = np.asarray(att_dst, _F32)
    res_W = np.asarray(res_W, _F32)
    batch = np.asarray(batch).astype(np.int64)

    # fused right matrix [F, 328] = [W.T | As.T | Ad.T | res_W.T]
    W3 = W.reshape(H, C, F)
    As = (att_src[:, :, None] * W3).sum(1)
    Ad = (att_dst[:, :, None] * W3).sum(1)
    Rcat = np.concatenate([W.T, As.T, Ad.T, res_W.T], axis=1).astype(_BF16)
    xT_bf = x.T.astype(_BF16)                        # [F, N]

    # ---- edges (+ self loops) ----
    loop = np.arange(N, dtype=np.int64)
    src = np.concatenate([np.asarray(edge_index[0]), loop]).astype(np.int64)
    dst = np.concatenate([np.asarray(edge_index[1]), loop]).astype(np.int64)
    deg = np.bincount(dst, minlength=N)
    n_core, n_tile, n_slot = _assign_nodes(deg)

    owner = n_core[dst]
    tl = n_tile[dst]
    dl = n_slot[dst]

    key = owner * TILES + tl
    counts = np.bincount(key, minlength=NCORE * TILES).reshape(NCORE, TILES)
    K_t = counts.max(axis=0).astype(np.int64)        # [TILES]
    nb_t = _cdiv(K_t, 128)
    K_pad = nb_t * 128

    order = np.lexsort((tl, owner))
    s_src, s_dl = src[order], dl[order]
    gstart = np.zeros(NCORE * TILES + 1, np.int64)
    gstart[1:] = np.cumsum(counts.flatten())

    # ---- static block bookkeeping (same on all cores) ----
    groups = [list(range(g0, min(g0 + G, TILES))) for g0 in range(0, TILES, G)]
    TOTBLK = int(nb_t.sum())
    tile_blocks = [[] for _ in range(TILES)]   # (global block idx, group col)
    seg_meta = []                              # per tile: (t, blk0)
    blk = 0
    group_B0 = []
    for gtiles in groups:
        group_B0.append(blk)
        xoff = 0
        for t in gtiles:
            nb = int(nb_t[t])
            seg_meta.append((t, blk))
            for k in range(nb):
                tile_blocks[t].append((blk, xoff + k * 128))
                blk += 1
            xoff += nb * 128
    assert blk == TOTBLK
    gb_per_group = [int(nb_t[np.array(g)].sum()) for g in groups]
    MAXGB = max(gb_per_group)
    MAXNST = max(len(tb) for tb in tile_blocks)

    own_all = []
    for k in range(NCORE):
        own = np.where(n_core == k)[0]
        own = own[np.argsort((n_tile[own] * 128 + n_slot[own]), kind="stable")]
        own_all.append(own)

    # ---- per-core tensors ----
    in_maps = []
    for k in range(NCORE):
        xedgeT = np.zeros((F, TOTBLK * 128), _BF16)
        dlflat = np.full(TOTBLK * 128, -1, np.int64)
        for (t, blk0) in seg_meta:
            gi = k * TILES + t
            n = int(counts[k, t])
            if n == 0:
                continue
            a = int(gstart[gi])
            sl = slice(blk0 * 128, blk0 * 128 + n)
            xedgeT[:, sl] = xT_bf[:, s_src[a:a + n]]
            dlflat[sl] = s_dl[a:a + n]
        dlb = dlflat.reshape(TOTBLK, 128)
        m1arr = (dlb[:, :, None] == np.arange(128)[None, None, :])
        m1_all = np.ascontiguousarray(
            m1arr.transpose(1, 0, 2).reshape(128, TOTBLK * 128)).astype(_FP8)
        m2t_all = np.ascontiguousarray(
            m1arr.transpose(2, 0, 1).reshape(128, TOTBLK * 128)).astype(_FP8)

        own = own_all[k]
        rowpos = n_tile[own] * 128 + n_slot[own]
        xTo = np.zeros((F, TILES * 128), _BF16)
        xTo[:, rowpos] = xT_bf[:, own]
        bown = batch[own]
        onehot_b = np.zeros((128, TILES * 8), _BF16)
        onehot_b[rowpos % 128, (rowpos // 128) * 8 + bown] = 1.0
        onehotT = np.zeros((8, TILES * 128), _F32)
        onehotT[bown, rowpos] = 1.0

        in_maps.append({
            "xedgeT": xedgeT, "Rcat": Rcat,
            "m1_all": m1_all, "m2t_all": m2t_all,
            "xTo": xTo, "onehot_b": onehot_b, "onehotT": onehotT,
        })

    bc_row = np.tile((np.asarray(bias_gat, _F32)
                      + np.asarray(res_b, _F32))[None, :], (128, 1))
    alpha_t = np.full((128, 1), NEG, _F32)
    gms = np.asarray(gn_mean_scale, _F32)
    cnt = np.bincount(batch, minlength=B).astype(_F32)
    gn_pack = np.zeros((8, 4 * C + 2), _F32)
    gn_pack[:, 0:C] = np.asarray(gn_weight, _F32)[None, :]
    gn_pack[:, C:2 * C] = np.asarray(gn_bias, _F32)[None, :]
    gn_pack[:, 2 * C:3 * C] = gms[None, :]
    gn_pack[:, 3 * C:4 * C] = (gms * (2.0 - gms))[None, :]
    gn_pack[:, 4 * C] = 1.0 / cnt
    gn_pack[:, 4 * C + 1] = EPS
    for m in in_maps:
        m.update({"bc_row": bc_row, "alpha_t": alpha_t, "gn_pack": gn_pack})

    cfg = {
        "groups": groups, "group_B0": group_B0, "gb_per_group": gb_per_group,
        "tile_blocks": tile_blocks, "TOTBLK": TOTBLK,
        "MAXGB": MAXGB, "MAXNST": MAXNST, "own_all": own_all, "nb_t": nb_t,
    }
    return cfg, in_maps


def _build_nc(cfg, debug=False):
    import concourse.bacc as bacc
    import concourse.mybir as mybir
    import concourse.tile as tile

    AF = mybir.ActivationFunctionType
    OP = mybir.AluOpType
    f32 = mybir.dt.float32
    bf16 = mybir.dt.bfloat16
    fp8 = mybir.dt.float8e4

    groups = cfg["groups"]
    group_B0 = cfg["group_B0"]
    gb_per_group = cfg["gb_per_group"]
    tile_blocks = cfg["tile_blocks"]
    TOTBLK = cfg["TOTBLK"]
    MAXGB, MAXNST = cfg["MAXGB"], cfg["MAXNST"]

    nc = bacc.Bacc("TRN2", target_bir_lowering=False)

    xedgeT = nc.declare_dram_parameter("xedgeT", [F, TOTBLK * 128], bf16, isOutput=False)
    Rcat = nc.declare_dram_parameter("Rcat", [F, 328], bf16, isOutput=False)
    m1_all = nc.declare_dram_parameter("m1_all", [128, TOTBLK * 128], fp8, isOutput=False)
    m2t_all = nc.declare_dram_parameter("m2t_all", [128, TOTBLK * 128], fp8, isOutput=False)
    xTo = nc.declare_dram_parameter("xTo", [F, TILES * 128], bf16, isOutput=False)
    onehot_b = nc.declare_dram_parameter("onehot_b", [128, TILES * 8], bf16, isOutput=False)
    onehotT = nc.declare_dram_parameter("onehotT", [8, TILES * 128], f32, isOutput=False)
    bc_row = nc.declare_dram_parameter("bc_row", [128, C], f32, isOutput=False)
    alpha_t = nc.declare_dram_parameter("alpha_t", [128, 1], f32, isOutput=False)
    gn_pack = nc.declare_dram_parameter("gn_pack", [8, 4 * C + 2], f32, isOutput=False)
    out = nc.declare_dram_parameter("out", [NOWN, C], f32, isOutput=True)

    cc_in = nc.dram_tensor("cc_in", [8, 2 * C], f32)
    cc_out = nc.dram_tensor("cc_out", [8, 2 * C], f32)
    if debug:
        dbg_h = nc.declare_dram_parameter("dbg_h", [128, TILES * C], f32, isOutput=True)
        dbg_adst = nc.declare_dram_parameter("dbg_adst", [128, TILES * 4], f32, isOutput=True)
        dbg_resid = nc.declare_dram_parameter("dbg_resid", [128, TILES * C], f32, isOutput=True)
        dbg_stats = nc.declare_dram_parameter("dbg_stats", [8, 2 * C], f32, isOutput=True)
        dbg_ex = nc.declare_dram_parameter("dbg_ex", [128, MAXNST * 4], f32, isOutput=True)

    with tile.TileContext(nc) as tc:
        with (
            tc.tile_pool(name="const", bufs=1) as cp,
            tc.tile_pool(name="persist", bufs=1) as pers,
            tc.tile_pool(name="xload", bufs=2) as xp,
            tc.tile_pool(name="xe", bufs=2) as xep,
            tc.tile_pool(name="m1pool", bufs=2) as mp1,
            tc.tile_pool(name="m2pool", bufs=2) as mp2,
            tc.tile_pool(name="rhsp", bufs=2) as rhp,
            tc.tile_pool(name="small", bufs=4) as smp,
        ):
            # ---- constants ----
            rc_sb = cp.tile([F, 328], bf16)
            nc.sync.dma_start(rc_sb[:], Rcat[:])
            bc_sb = cp.tile([128, C], f32)
            nc.sync.dma_start(bc_sb[:], bc_row[:])
            ohb_sb = cp.tile([128, TILES * 8], bf16)
            nc.sync.dma_start(ohb_sb[:], onehot_b[:])
            al_sb = cp.tile([128, 1], f32)
            nc.sync.dma_start(al_sb[:], alpha_t[:])
            gn_sb = cp.tile([8, 4 * C + 2], f32)
            nc.sync.dma_start(gn_sb[:], gn_pack[:])


            adst_sb = pers.tile([128, TILES * 4], f32)
            resid_sb = pers.tile([128, TILES * C], bf16)
            h_sb = pers.tile([128, TILES * C], bf16)

            with tc.tile_pool(name="psum1", bufs=3, space="PSUM") as ps1:
                # ---- phase 1: owned-node sweep -> a_dst + residual ----
                for g0 in range(0, TILES, G):
                    ng = min(G, TILES - g0)
                    xs = xp.tile([F, G * 128], bf16, tag="xo")
                    nc.sync.dma_start(xs[:, 0:ng * 128],
                                      xTo[:, g0 * 128:(g0 + ng) * 128])
                    for i in range(ng):
                        t = g0 + i
                        ps = ps1.tile([128, 68], f32, tag="ops")
                        nc.tensor.matmul(ps[:], lhsT=xs[:, i * 128:(i + 1) * 128],
                                         rhs=rc_sb[:, 260:328],
                                         start=True, stop=True)
                        nc.vector.tensor_copy(adst_sb[:, t * 4:(t + 1) * 4],
                                              ps[:, 0:4])
                        nc.vector.tensor_tensor(
                            out=resid_sb[:, t * C:(t + 1) * C],
                            in0=ps[:, 4:68], in1=bc_sb[:], op=OP.add)

            # ---- phase 2: edge sweep ----
            with (
                tc.tile_pool(name="psum_xl", bufs=2, space="PSUM") as pxl,
                tc.tile_pool(name="psum_agg", bufs=2, space="PSUM") as pag,
                tc.tile_pool(name="psum_stat", bufs=1, space="PSUM") as pst,
            ):
                stats_ps = pst.tile([8, 2 * C], f32)
                duo_ctr = [0]
                for gi, gtiles in enumerate(groups):
                    B0 = group_B0[gi]
                    GB = gb_per_group[gi]
                    xe = xep.tile([F, MAXGB * 128], bf16, tag="xe")
                    nc.sync.dma_start(xe[:, 0:GB * 128],
                                      xedgeT[:, B0 * 128:(B0 + GB) * 128])
                    m1s = mp1.tile([128, MAXGB * 128], fp8, tag="m1")
                    nc.sync.dma_start(m1s[:, 0:GB * 128],
                                      m1_all[:, B0 * 128:(B0 + GB) * 128])
                    m2s = mp2.tile([128, MAXGB * 128], fp8, tag="m2")
                    nc.sync.dma_start(m2s[:, 0:GB * 128],
                                      m2t_all[:, B0 * 128:(B0 + GB) * 128])

                    for t in gtiles:
                        blocks = tile_blocks[t]
                        nst = len(blocks)
                        adstb = smp.tile([128, 4], bf16, tag="adstb")
                        nc.vector.tensor_copy(adstb[:],
                                              adst_sb[:, t * 4:(t + 1) * 4])
                        ex32 = smp.tile([128, MAXNST * 4], f32, tag="ex32")
                        rhs_t = rhp.tile([128, MAXNST * 260], bf16, tag="rhs")
                        agg = pag.tile([128, 260], f32, tag="agg")
                        for d0 in range(0, nst, 2):
                            nd = min(2, nst - d0)
                            # two blocks per psum tile at 512-col offsets so
                            # each block's 260 cols stay within one 2KB bank
                            ps_xl = pxl.tile([128, 1024], f32, tag="xlps")
                            for u in range(nd):
                                bj = d0 + u
                                Bg, gcol = blocks[bj]
                                mb = (Bg - B0) * 128
                                nc.tensor.matmul(
                                    ps_xl[:, u * 512:u * 512 + 260],
                                    lhsT=xe[:, gcol:gcol + 128],
                                    rhs=rc_sb[:, 0:260],
                                    start=True, stop=False,
                                    skip_group_check=True)
                                nc.tensor.matmul(
                                    ps_xl[:, u * 512 + 256:u * 512 + 260],
                                    lhsT=m2s[:, mb:mb + 128],
                                    rhs=adstb[:], start=False, stop=True,
                                    skip_group_check=True)
                            exs = ex32[:, d0 * 4:(d0 + nd) * 4]
                            nc.scalar.activation(
                                out=exs.rearrange("p (u h) -> p u h", h=4),
                                in_=ps_xl[:].rearrange(
                                    "p (u x) -> p u x", x=512)[:, 0:nd, 256:260],
                                func=AF.Prelu, alpha=al_sb[:, 0:1])
                            nc.scalar.activation(out=exs, in_=exs, func=AF.Exp)
                            if duo_ctr[0] % DVE_MOD < DVE_CUT:
                                nc.vector.tensor_tensor(
                                    out=rhs_t[:, d0 * 260:(d0 + nd) * 260]
                                    .rearrange("p (j x) -> p j x", x=260)
                                    [:, :, 0:256]
                                    .rearrange("p j (h c) -> p j h c", h=H),
                                    in0=ps_xl[:].rearrange(
                                        "p (j x) -> p j x", x=512)[:, 0:nd, 0:256]
                                    .rearrange("p j (h c) -> p j h c", h=H),
                                    in1=exs.rearrange("p (j h) -> p j h", h=4)
                                    .to_broadcast([128, nd, H, C]),
                                    op=OP.mult)
                            else:
                                for u in range(nd):
                                    bj = d0 + u
                                    for h in range(H):
                                        nc.scalar.activation(
                                            out=rhs_t[:, bj * 260 + h * C:
                                                      bj * 260 + (h + 1) * C],
                                            in_=ps_xl[:, u * 512 + h * C:
                                                      u * 512 + (h + 1) * C],
                                            func=AF.Copy,
                                            scale=ex32[:, bj * 4 + h:
                                                       bj * 4 + h + 1])
                            duo_ctr[0] += 1
                            nc.vector.tensor_copy(
                                rhs_t[:, d0 * 260:(d0 + nd) * 260]
                                .rearrange("p (j x) -> p j x", x=260)
                                [:, :, 256:260],
                                exs.rearrange("p (j h) -> p j h", h=4))
                            for u in range(nd):
                                bj = d0 + u
                                Bg, gcol = blocks[bj]
                                mb = (Bg - B0) * 128
                                nc.tensor.matmul(
                                    agg[:], lhsT=m1s[:, mb:mb + 128],
                                    rhs=rhs_t[:, bj * 260:(bj + 1) * 260],
                                    start=(bj == 0), stop=(bj == nst - 1))
                        if debug and t == 0:
                            nc.sync.dma_start(dbg_ex[:, 0:nst * 4],
                                              ex32[:, 0:nst * 4])
                        # combine heads, add residual
                        dn = smp.tile([128, 4], f32, tag="dn")
                        nc.vector.tensor_scalar(
                            out=dn[:], in0=agg[:, 256:260], scalar1=1e-6,
                            scalar2=None, op0=OP.add)
                        recip = smp.tile([128, 4], f32, tag="recip")
                        nc.vector.reciprocal(recip[:], dn[:])
                        hacc = smp.tile([128, C], f32, tag="hacc")
                        nc.vector.tensor_scalar(
                            out=hacc[:], in0=agg[:, 0:C], scalar1=recip[:, 0:1],
                            scalar2=None, op0=OP.mult)
                        for h in range(1, H):
                            nc.vector.scalar_tensor_tensor(
                                out=hacc[:], in0=agg[:, h * C:(h + 1) * C],
                                scalar=recip[:, h:h + 1], in1=hacc[:],
                                op0=OP.mult, op1=OP.add)
                        hsl = h_sb[:, t * C:(t + 1) * C]
                        nc.vector.scalar_tensor_tensor(
                            out=hsl, in0=hacc[:], scalar=1.0 / H,
                            in1=resid_sb[:, t * C:(t + 1) * C],
                            op0=OP.mult, op1=OP.add)
                        # graphnorm partial stats (accumulate in psum)
                        sq = smp.tile([128, C], bf16, tag="sq")
                        nc.scalar.square(sq[:], hsl)
                        nc.tensor.matmul(stats_ps[:, 0:C],
                                         lhsT=ohb_sb[:, t * 8:(t + 1) * 8],
                                         rhs=hsl, start=(t == 0),
                                         stop=(t == TILES - 1),
                                         skip_group_check=True)
                        nc.tensor.matmul(stats_ps[:, C:2 * C],
                                         lhsT=ohb_sb[:, t * 8:(t + 1) * 8],
                                         rhs=sq[:], start=(t == 0),
                                         stop=(t == TILES - 1),
                                         skip_group_check=True)
                stats_sb = pers.tile([8, 2 * C], f32)
                nc.vector.tensor_copy(stats_sb[:], stats_ps[:])
            if debug:
                nc.sync.dma_start(dbg_adst[:], adst_sb[:])
                nc.sync.dma_start(dbg_stats[:], stats_sb[:])
                nc.gpsimd.dma_start(dbg_resid[:], resid_sb[:])
                nc.gpsimd.dma_start(dbg_h[:], h_sb[:])

            # ---- phase 3: AllReduce stats, normalize, gelu, write out ----
            with tc.tile_pool(name="psum3", bufs=2, space="PSUM") as ps3, \
                 tc.tile_pool(name="ohtp", bufs=2) as ohp:
                nc.gpsimd.dma_start(cc_in[:], stats_sb[:])
                nc.gpsimd.collective_compute(
                    "AllReduce", OP.add,
                    replica_groups=[list(range(NCORE))],
                    ins=[cc_in[:]], outs=[cc_out[:]])
                sall = smp.tile([8, 2 * C], f32, tag="sall")
                nc.sync.dma_start(sall[:], cc_out[:])
                gw = gn_sb[:, 0:C]
                gb = gn_sb[:, C:2 * C]
                gms = gn_sb[:, 2 * C:3 * C]
                gms2m = gn_sb[:, 3 * C:4 * C]
                invc = gn_sb[:, 4 * C:4 * C + 1]
                epsc = gn_sb[:, 4 * C + 1:4 * C + 2]
                mean = smp.tile([8, C], f32, tag="mean")
                nc.vector.tensor_scalar(out=mean[:], in0=sall[:, 0:C],
                                        scalar1=invc, scalar2=None, op0=OP.mult)
                eh2 = smp.tile([8, C], f32, tag="eh2")
                nc.vector.tensor_scalar(out=eh2[:], in0=sall[:, C:2 * C],
                                        scalar1=invc, scalar2=None, op0=OP.mult)
                msq = smp.tile([8, C], f32, tag="msq")
                nc.vector.tensor_tensor(out=msq[:], in0=mean[:], in1=mean[:],
                                        op=OP.mult)
                var = smp.tile([8, C], f32, tag="var")
                nc.vector.tensor_tensor(out=msq[:], in0=msq[:], in1=gms2m,
                                        op=OP.mult)
                nc.vector.tensor_tensor(out=var[:], in0=eh2[:], in1=msq[:],
                                        op=OP.subtract)
                std = smp.tile([8, C], f32, tag="std")
                nc.scalar.activation(out=std[:], in_=var[:], func=AF.Sqrt,
                                     bias=epsc)
                ab = smp.tile([8, 2 * C], f32, tag="ab")
                nc.vector.reciprocal(std[:], std[:])
                nc.vector.tensor_tensor(out=ab[:, 0:C], in0=gw, in1=std[:],
                                        op=OP.mult)
                tm = smp.tile([8, C], f32, tag="tm")
                nc.vector.tensor_tensor(out=tm[:], in0=ab[:, 0:C], in1=mean[:],
                                        op=OP.mult)
                nc.vector.tensor_tensor(out=tm[:], in0=tm[:], in1=gms,
                                        op=OP.mult)
                nc.vector.tensor_tensor(out=ab[:, C:2 * C], in0=gb, in1=tm[:],
                                        op=OP.subtract)

                for g0 in range(0, TILES, G):
                    ng = min(G, TILES - g0)
                    obuf = ohp.tile([128, G * C], f32, tag="ob")
                    oht = ohp.tile([8, G * 128], f32, tag="oht")
                    nc.sync.dma_start(oht[:, 0:ng * 128],
                                      onehotT[:, g0 * 128:(g0 + ng) * 128])
                    for i in range(ng):
                        t = g0 + i
                        abpe = ps3.tile([128, 2 * C], f32, tag="abpe")
                        nc.tensor.matmul(abpe[:],
                                         lhsT=oht[:, i * 128:(i + 1) * 128],
                                         rhs=ab[:], start=True, stop=True)
                        nrm = smp.tile([128, C], f32, tag="nrm")
                        nc.vector.tensor_tensor(out=nrm[:],
                                                in0=h_sb[:, t * C:(t + 1) * C],
                                                in1=abpe[:, 0:C], op=OP.mult)
                        nc.vector.tensor_tensor(out=nrm[:], in0=nrm[:],
                                                in1=abpe[:, C:2 * C], op=OP.add)
                        nc.scalar.activation(out=obuf[:, i * C:(i + 1) * C],
                                             in_=nrm[:],
                                             func=AF.Gelu_apprx_tanh)
                    nfull = ng if g0 + ng < TILES else ng - 1
                    if nfull > 0:
                        nc.sync.dma_start(
                            out[g0 * 128:(g0 + nfull) * 128, :]
                            .rearrange("(g p) c -> p g c", p=128),
                            obuf[:, 0:nfull * C]
                            .rearrange("p (g c) -> p g c", c=C))
                    if g0 + ng == TILES:
                        nc.sync.dma_start(
                            out[(TILES - 1) * 128:(TILES - 1) * 128 + LAST_ROWS, :],
                            obuf[0:LAST_ROWS, (ng - 1) * C:ng * C])

    nc.compile()
    return nc


def kernel(**inputs):
    from concourse.bass_utils import run_bass_kernel_spmd

    cfg, in_maps = _host_prep(**inputs)
    nc = _build_nc(cfg)
    res = run_bass_kernel_spmd(nc, in_maps, core_ids=list(range(NCORE)))
    full = np.empty((N, C), _F32)
    for k in range(NCORE):
        full[cfg["own_all"][k]] = res.results[k]["out"]
    return full


# revision 31
# speedup vs baseline: 1.4816x; 1.4816x over previous
"""Multi-head GAT layer (4 heads, mean-aggregated) + residual + GraphNorm + gelu
on 8 Trainium2 NeuronCores (SPMD, one NEFF on all cores).

v3 strategy:
  - dst nodes are dealt to (core, tile, slot) round-robin by in-degree so per
    tile edge counts are balanced across cores (less SPMD max-padding).
  - NO on-device gather (SWDGE descriptor generation is serial on the Pool
    engine at ~2-8ns/descriptor -> ~1ms floor for per-edge gathers).  Instead
    the host lays out the per-edge source rows x[src_e] directly in transposed
    block-aligned order (xedgeT[f, edge_slot]) and the kernel streams them as
    bulk contiguous DMA.  Edge slots are grouped as 98 dst tiles x ~13 blocks
    of 128 edges.
  - Per 128-edge block, PE recomputes xl = x@W.T (and a_src = x@As.T); the
    one-hot-distributed a_dst (ape) accumulates onto a_src in one PSUM tile
    per dst tile, so leakyrelu+exp run batched per tile on ACT.
  - One-hot matrices (m1: [e,d], m2t: [d,e]) are uploaded as fp8 (exact 0/1)
    and used directly as matmul weights against bf16 moving operands.
  - The per-edge coefficient multiply (rhs = xl * ex) runs on two-block PSUM
    tiles and is split between DVE and ACT.
  - Softmax denominators ride as 4 extra columns of the agg matmul rhs.
  - GraphNorm stats accumulate in a persistent PSUM bank across all tiles,
    then AllReduce across the 8 cores.
"""
import numpy as np
import ml_dtypes

_BF16 = ml_dtypes.bfloat16
_FP8 = ml_dtypes.float8_e4m3
_F32 = np.float32

N, F, C, H, E, B = 100000, 128, 64, 4, 1200000, 8
NCORE = 8
NEG = 0.2
EPS = 1e-5
NOWN = N // NCORE             # 12500 dst nodes per core
TILES = (NOWN + 127) // 128   # 98 dst tiles per core (last has 84 slots)
LAST_ROWS = NOWN - (TILES - 1) * 128
G = 7                         # dst tiles per DMA group (98 = 14*7)
DVE_MOD = 8                   # duos with (idx % DVE_MOD) < DVE_CUT multiply on DVE
DVE_CUT = 7


def _cdiv(a, b):
    return (a + b - 1) // b


def _assign_nodes(deg):
    """Deal nodes (by descending degree) round-robin over the 784 (core,tile)
    pairs, honoring per-tile capacity. Returns [N] arrays core, tile, slot."""
    P = NCORE * TILES
    caps = np.full(P, 128, np.int64)
    caps[TILES - 1::TILES] = LAST_ROWS
    order = np.argsort(-deg, kind="stable")
    fill = np.zeros(P, np.int64)
    core = np.empty(N, np.int64)
    tile = np.empty(N, np.int64)
    slot = np.empty(N, np.int64)
    p = 0
    for n in order:
        while fill[p] >= caps[p]:
            p = (p + 1) % P
        core[n] = p // TILES
        tile[n] = p % TILES
        slot[n] = fill[p]
        fill[p] += 1
        p = (p + 1) % P
    return core, tile, slot


def _host_prep(x, edge_index, batch, W, att_src, att_dst, bias_gat, res_W,
               res_b, gn_weight, gn_bias, gn_mean_scale):
    x = np.asarray(x, _F32)
    W = np.asarray(W, _F32)
    att_src = np.asarray(att_src, _F32)
    att_dst = np.asarray(att_dst, _F32)
    res_W = np.asarray(res_W, _F32)
    batch = np.asarray(batch).astype(np.int64)

    # fused right matrix [F, 328] = [W.T | As.T | Ad.T | res_W.T]
    W3 = W.reshape(H, C, F)
    As = (att_src[:, :, None] * W3).sum(1)
    Ad = (att_dst[:, :, None] * W3).sum(1)
    Rcat = np.concatenate([W.T, As.T, Ad.T, res_W.T], axis=1).astype(_BF16)
    xT_bf = x.T.astype(_BF16)                        # [F, N]

    # ---- edges (+ self loops) ----
    loop = np.arange(N, dtype=np.int64)
    src = np.concatenate([np.asarray(edge_index[0]), loop]).astype(np.int64)
    dst = np.concatenate([np.asarray(edge_index[1]), loop]).astype(np.int64)
    deg = np.bincount(dst, minlength=N)
    n_core, n_tile, n_slot = _assign_nodes(deg)

    owner = n_core[dst]
    tl = n_tile[dst]
    dl = n_slot[dst]

    key = owner * TILES + tl
    counts = np.bincount(key, minlength=NCORE * TILES).reshape(NCORE, TILES)
    K_t = counts.max(axis=0).astype(np.int64)        # [TILES]
    nb_t = _cdiv(K_t, 128)
    K_pad = nb_t * 128

    order = np.lexsort((tl, owner))
    s_src, s_dl = src[order], dl[order]
    gstart = np.zeros(NCORE * TILES + 1, np.int64)
    gstart[1:] = np.cumsum(counts.flatten())

    # ---- static block bookkeeping (same on all cores) ----
    groups = [list(range(g0, min(g0 + G, TILES))) for g0 in range(0, TILES, G)]
    TOTBLK = int(nb_t.sum())
    tile_blocks = [[] for _ in range(TILES)]   # (global block idx, group col)
    seg_meta = []                              # per tile: (t, blk0)
    blk = 0
    group_B0 = []
    for gtiles in groups:
        group_B0.append(blk)
        xoff = 0
        for t in gtiles:
            nb = int(nb_t[t])
            seg_meta.append((t, blk))
            for k in range(nb):
                tile_blocks[t].append((blk, xoff + k * 128))
                blk += 1
            xoff += nb * 128
    assert blk == TOTBLK
    gb_per_group = [int(nb_t[np.array(g)].sum()) for g in groups]
    MAXGB = max(gb_per_group)
    MAXNST = max(len(tb) for tb in tile_blocks)

    own_all = []
    for k in range(NCORE):
        own = np.where(n_core == k)[0]
        own = own[np.argsort((n_tile[own] * 128 + n_slot[own]), kind="stable")]
        own_all.append(own)

    # ---- per-core tensors ----
    in_maps = []
    for k in range(NCORE):
        xedgeT = np.zeros((F, TOTBLK * 128), _BF16)
        dlflat = np.full(TOTBLK * 128, -1, np.int64)
        for (t, blk0) in seg_meta:
            gi = k * TILES + t
            n = int(counts[k, t])
            if n == 0:
                continue
            a = int(gstart[gi])
            sl = slice(blk0 * 128, blk0 * 128 + n)
            xedgeT[:, sl] = xT_bf[:, s_src[a:a + n]]
            dlflat[sl] = s_dl[a:a + n]
        dlb = dlflat.reshape(TOTBLK, 128)
        m1arr = (dlb[:, :, None] == np.arange(128)[None, None, :])
        m1_all = np.ascontiguousarray(
            m1arr.transpose(1, 0, 2).reshape(128, TOTBLK * 128)).astype(_FP8)
        m2t_all = np.ascontiguousarray(
            m1arr.transpose(2, 0, 1).reshape(128, TOTBLK * 128)).astype(_FP8)

        own = own_all[k]
        rowpos = n_tile[own] * 128 + n_slot[own]
        xTo = np.zeros((F, TILES * 128), _BF16)
        xTo[:, rowpos] = xT_bf[:, own]
        bown = batch[own]
        onehot_b = np.zeros((128, TILES * 8), _BF16)
        onehot_b[rowpos % 128, (rowpos // 128) * 8 + bown] = 1.0
        onehotT = np.zeros((8, TILES * 128), _F32)
        onehotT[bown, rowpos] = 1.0

        in_maps.append({
            "xedgeT": xedgeT, "Rcat": Rcat,
            "m1_all": m1_all, "m2t_all": m2t_all,
            "xTo": xTo, "onehot_b": onehot_b, "onehotT": onehotT,
        })

    bc_row = np.tile((np.asarray(bias_gat, _F32)
                      + np.asarray(res_b, _F32))[None, :], (128, 1))
    alpha_t = np.full((128, 1), NEG, _F32)
    gms = np.asarray(gn_mean_scale, _F32)
    cnt = np.bincount(batch, minlength=B).astype(_F32)
    gn_pack = np.zeros((8, 4 * C + 2), _F32)
    gn_pack[:, 0:C] = np.asarray(gn_weight, _F32)[None, :]
    gn_pack[:, C:2 * C] = np.asarray(gn_bias, _F32)[None, :]
    gn_pack[:, 2 * C:3 * C] = gms[None, :]
    gn_pack[:, 3 * C:4 * C] = (gms * (2.0 - gms))[None, :]
    gn_pack[:, 4 * C] = 1.0 / cnt
    gn_pack[:, 4 * C + 1] = EPS
    for m in in_maps:
        m.update({"bc_row": bc_row, "alpha_t": alpha_t, "gn_pack": gn_pack})

    cfg = {
        "groups": groups, "group_B0": group_B0, "gb_per_group": gb_per_group,
        "tile_blocks": tile_blocks, "TOTBLK": TOTBLK,
        "MAXGB": MAXGB, "MAXNST": MAXNST, "own_all": own_all, "nb_t": nb_t,
    }
    return cfg, in_maps


def _build_nc(cfg, debug=False):
    import concourse.bacc as bacc
    import concourse.mybir as mybir
    import concourse.tile as tile

    AF = mybir.ActivationFunctionType
    OP = mybir.AluOpType
    f32 = mybir.dt.float32
    bf16 = mybir.dt.bfloat16
    fp8 = mybir.dt.float8e4

    groups = cfg["groups"]
    group_B0 = cfg["group_B0"]
    gb_per_group = cfg["gb_per_group"]
    tile_blocks = cfg["tile_blocks"]
    TOTBLK = cfg["TOTBLK"]
    MAXGB, MAXNST = cfg["MAXGB"], cfg["MAXNST"]

    nc = bacc.Bacc("TRN2", target_bir_lowering=False)

    xedgeT = nc.declare_dram_parameter("xedgeT", [F, TOTBLK * 128], bf16, isOutput=False)
    Rcat = nc.declare_dram_parameter("Rcat", [F, 328], bf16, isOutput=False)
    m1_all = nc.declare_dram_parameter("m1_all", [128, TOTBLK * 128], fp8, isOutput=False)
    m2t_all = nc.declare_dram_parameter("m2t_all", [128, TOTBLK * 128], fp8, isOutput=False)
    xTo = nc.declare_dram_parameter("xTo", [F, TILES * 128], bf16, isOutput=False)
    onehot_b = nc.declare_dram_parameter("onehot_b", [128, TILES * 8], bf16, isOutput=False)
    onehotT = nc.declare_dram_parameter("onehotT", [8, TILES * 128], f32, isOutput=False)
    bc_row = nc.declare_dram_parameter("bc_row", [128, C], f32, isOutput=False)
    alpha_t = nc.declare_dram_parameter("alpha_t", [128, 1], f32, isOutput=False)
    gn_pack = nc.declare_dram_parameter("gn_pack", [8, 4 * C + 2], f32, isOutput=False)
    out = nc.declare_dram_parameter("out", [NOWN, C], f32, isOutput=True)

    cc_in = nc.dram_tensor("cc_in", [8, 2 * C], f32)
    cc_out = nc.dram_tensor("cc_out", [8, 2 * C], f32)
    if debug:
        dbg_h = nc.declare_dram_parameter("dbg_h", [128, TILES * C], f32, isOutput=True)
        dbg_adst = nc.declare_dram_parameter("dbg_adst", [128, TILES * 4], f32, isOutput=True)
        dbg_resid = nc.declare_dram_parameter("dbg_resid", [128, TILES * C], f32, isOutput=True)
        dbg_stats = nc.declare_dram_parameter("dbg_stats", [8, 2 * C], f32, isOutput=True)
        dbg_ex = nc.declare_dram_parameter("dbg_ex", [128, MAXNST * 4], f32, isOutput=True)

    with tile.TileContext(nc) as tc:
        with (
            tc.tile_pool(name="const", bufs=1) as cp,
            tc.tile_pool(name="persist", bufs=1) as pers,
            tc.tile_pool(name="xload", bufs=2) as xp,
            tc.tile_pool(name="xe", bufs=2) as xep,
            tc.tile_pool(name="m1pool", bufs=2) as mp1,
            tc.tile_pool(name="m2pool", bufs=2) as mp2,
            tc.tile_pool(name="rhsp", bufs=2) as rhp,
            tc.tile_pool(name="small", bufs=4) as smp,
        ):
            # ---- constants ----
            rc_sb = cp.tile([F, 328], bf16)
            nc.sync.dma_start(rc_sb[:], Rcat[:])
            bc_sb = cp.tile([128, C], f32)
            nc.sync.dma_start(bc_sb[:], bc_row[:])
            ohb_sb = cp.tile([128, TILES * 8], bf16)
            nc.sync.dma_start(ohb_sb[:], onehot_b[:])
            al_sb = cp.tile([128, 1], f32)
            nc.sync.dma_start(al_sb[:], alpha_t[:])
            gn_sb = cp.tile([8, 4 * C + 2], f32)
            nc.sync.dma_start(gn_sb[:], gn_pack[:])


            adst_sb = pers.tile([128, TILES * 4], f32)
            resid_sb = pers.tile([128, TILES * C], bf16)
            h_sb = pers.tile([128, TILES * C], bf16)

            with tc.tile_pool(name="psum1", bufs=3, space="PSUM") as ps1:
                # ---- phase 1: owned-node sweep -> a_dst + residual ----
                for g0 in range(0, TILES, G):
                    ng = min(G, TILES - g0)
                    xs = xp.tile([F, G * 128], bf16, tag="xo")
                    nc.sync.dma_start(xs[:, 0:ng * 128],
                                      xTo[:, g0 * 128:(g0 + ng) * 128])
                    for i in range(ng):
                        t = g0 + i
                        ps = ps1.tile([128, 68], f32, tag="ops")
                        nc.tensor.matmul(ps[:], lhsT=xs[:, i * 128:(i + 1) * 128],
                                         rhs=rc_sb[:, 260:328],
                                         start=True, stop=True)
                        nc.vector.tensor_copy(adst_sb[:, t * 4:(t + 1) * 4],
                                              ps[:, 0:4])
                        nc.vector.tensor_tensor(
                            out=resid_sb[:, t * C:(t + 1) * C],
                            in0=ps[:, 4:68], in1=bc_sb[:], op=OP.add)

            # ---- phase 2: edge sweep ----
            with (
                tc.tile_pool(name="psum_xl", bufs=3, space="PSUM") as pxl,
                tc.tile_pool(name="psum_lr", bufs=2, space="PSUM") as plr,
                tc.tile_pool(name="psum_agg", bufs=2, space="PSUM") as pag,
                tc.tile_pool(name="psum_stat", bufs=1, space="PSUM") as pst,
            ):
                stats_ps = pst.tile([8, 2 * C], f32)
                duo_ctr = [0]
                for gi, gtiles in enumerate(groups):
                    B0 = group_B0[gi]
                    GB = gb_per_group[gi]
                    xe = xep.tile([F, MAXGB * 128], bf16, tag="xe")
                    nc.sync.dma_start(xe[:, 0:GB * 128],
                                      xedgeT[:, B0 * 128:(B0 + GB) * 128])
                    m1s = mp1.tile([128, MAXGB * 128], fp8, tag="m1")
                    nc.sync.dma_start(m1s[:, 0:GB * 128],
                                      m1_all[:, B0 * 128:(B0 + GB) * 128])
                    m2s = mp2.tile([128, MAXGB * 128], fp8, tag="m2")
                    nc.sync.dma_start(m2s[:, 0:GB * 128],
                                      m2t_all[:, B0 * 128:(B0 + GB) * 128])

                    for t in gtiles:
                        blocks = tile_blocks[t]
                        nst = len(blocks)
                        adstb = smp.tile([128, 4], bf16, tag="adstb")
                        nc.vector.tensor_copy(adstb[:],
                                              adst_sb[:, t * 4:(t + 1) * 4])
                        # a_src + distributed a_dst -> lr (one psum tile)
                        ps_lr = plr.tile([128, MAXNST * 4], f32, tag="lr")
                        for bj, (Bg, gcol) in enumerate(blocks):
                            xel = xe[:, gcol:gcol + 128]
                            nc.tensor.matmul(
                                ps_lr[:, bj * 4:(bj + 1) * 4], lhsT=xel,
                                rhs=rc_sb[:, 256:260], start=True, stop=False,
                                skip_group_check=True)
                            mb = (Bg - B0) * 128
                            nc.tensor.matmul(
                                ps_lr[:, bj * 4:(bj + 1) * 4],
                                lhsT=m2s[:, mb:mb + 128],
                                rhs=adstb[:], start=False, stop=True,
                                skip_group_check=True)
                        ex32 = smp.tile([128, MAXNST * 4], f32, tag="ex32")
                        nc.scalar.activation(out=ex32[:, 0:nst * 4],
                                             in_=ps_lr[:, 0:nst * 4],
                                             func=AF.Prelu, alpha=al_sb[:, 0:1])
                        nc.scalar.activation(out=ex32[:, 0:nst * 4],
                                             in_=ex32[:, 0:nst * 4], func=AF.Exp)
                        if debug and t == 0:
                            nc.sync.dma_start(dbg_ex[:, 0:nst * 4],
                                              ex32[:, 0:nst * 4])
                        rhs_t = rhp.tile([128, MAXNST * 260], bf16, tag="rhs")
                        nc.vector.tensor_copy(
                            rhs_t[:, 0:nst * 260]
                            .rearrange("p (j x) -> p j x", x=260)[:, :, 256:260],
                            ex32[:, 0:nst * 4]
                            .rearrange("p (j h) -> p j h", h=4))
                        agg = pag.tile([128, 260], f32, tag="agg")
                        for d0 in range(0, nst, 2):
                            nd = min(2, nst - d0)
                            ps_xl = pxl.tile([128, 512], f32, tag="xlps")
                            for u in range(nd):
                                bj = d0 + u
                                Bg, gcol = blocks[bj]
                                nc.tensor.matmul(
                                    ps_xl[:, u * 256:(u + 1) * 256],
                                    lhsT=xe[:, gcol:gcol + 128],
                                    rhs=rc_sb[:, 0:256],
                                    start=True, stop=True,
                                    skip_group_check=True)
                            if duo_ctr[0] % DVE_MOD < DVE_CUT:
                                nc.vector.tensor_tensor(
                                    out=rhs_t[:, d0 * 260:(d0 + nd) * 260]
                                    .rearrange("p (j x) -> p j x", x=260)
                                    [:, :, 0:256]
                                    .rearrange("p j (h c) -> p j h c", h=H),
                                    in0=ps_xl[:, 0:nd * 256]
                                    .rearrange("p (j h c) -> p j h c", h=H, c=C),
                                    in1=ex32[:, d0 * 4:(d0 + nd) * 4]
                                    .rearrange("p (j h) -> p j h", h=4)
                                    .to_broadcast([128, nd, H, C]),
                                    op=OP.mult)
                            else:
                                for u in range(nd):
                                    bj = d0 + u
                                    for h in range(H):
                                        nc.scalar.activation(
                                            out=rhs_t[:, bj * 260 + h * C:
                                                      bj * 260 + (h + 1) * C],
                                            in_=ps_xl[:, u * 256 + h * C:
                                                      u * 256 + (h + 1) * C],
                                            func=AF.Copy,
                                            scale=ex32[:, bj * 4 + h:
                                                       bj * 4 + h + 1])
                            duo_ctr[0] += 1
                            for u in range(nd):
                                bj = d0 + u
                                Bg, gcol = blocks[bj]
                                mb = (Bg - B0) * 128
                                nc.tensor.matmul(
                                    agg[:], lhsT=m1s[:, mb:mb + 128],
                                    rhs=rhs_t[:, bj * 260:(bj + 1) * 260],
                                    start=(bj == 0), stop=(bj == nst - 1))
                        # combine heads, add residual
                        dn = smp.tile([128, 4], f32, tag="dn")
                        nc.vector.tensor_scalar(
                            out=dn[:], in0=agg[:, 256:260], scalar1=1e-6,
                            scalar2=None, op0=OP.add)
                        recip = smp.tile([128, 4], f32, tag="recip")
                        nc.vector.reciprocal(recip[:], dn[:])
                        hacc = smp.tile([128, C], f32, tag="hacc")
                        nc.vector.tensor_scalar(
                            out=hacc[:], in0=agg[:, 0:C], scalar1=recip[:, 0:1],
                            scalar2=None, op0=OP.mult)
                        for h in range(1, H):
                            nc.vector.scalar_tensor_tensor(
                                out=hacc[:], in0=agg[:, h * C:(h + 1) * C],
                                scalar=recip[:, h:h + 1], in1=hacc[:],
                                op0=OP.mult, op1=OP.add)
                        hsl = h_sb[:, t * C:(t + 1) * C]
                        nc.vector.scalar_tensor_tensor(
                            out=hsl, in0=hacc[:], scalar=1.0 / H,
                            in1=resid_sb[:, t * C:(t + 1) * C],
                            op0=OP.mult, op1=OP.add)
                        # graphnorm partial stats (accumulate in psum)
                        sq = smp.tile([128, C], bf16, tag="sq")
                        nc.scalar.square(sq[:], hsl)
                        nc.tensor.matmul(stats_ps[:, 0:C],
                                         lhsT=ohb_sb[:, t * 8:(t + 1) * 8],
                                         rhs=hsl, start=(t == 0),
                                         stop=(t == TILES - 1),
                                         skip_group_check=True)
                        nc.tensor.matmul(stats_ps[:, C:2 * C],
                                         lhsT=ohb_sb[:, t * 8:(t + 1) * 8],
                                         rhs=sq[:], start=(t == 0),
                                         stop=(t == TILES - 1),
                                         skip_group_check=True)
                stats_sb = pers.tile([8, 2 * C], f32)
                nc.vector.tensor_copy(stats_sb[:], stats_ps[:])
            if debug:
                nc.sync.dma_start(dbg_adst[:], adst_sb[:])
                nc.sync.dma_start(dbg_stats[:], stats_sb[:])
                nc.gpsimd.dma_start(dbg_resid[:], resid_sb[:])
                nc.gpsimd.dma_start(dbg_h[:], h_sb[:])

            # ---- phase 3: AllReduce stats, normalize, gelu, write out ----
            with tc.tile_pool(name="psum3", bufs=2, space="PSUM") as ps3, \
                 tc.tile_pool(name="ohtp", bufs=2) as ohp:
                nc.gpsimd.dma_start(cc_in[:], stats_sb[:])
                nc.gpsimd.collective_compute(
                    "AllReduce", OP.add,
                    replica_groups=[list(range(NCORE))],
                    ins=[cc_in[:]], outs=[cc_out[:]])
                sall = smp.tile([8, 2 * C], f32, tag="sall")
                nc.sync.dma_start(sall[:], cc_out[:])
                gw = gn_sb[:, 0:C]
                gb = gn_sb[:, C:2 * C]
                gms = gn_sb[:, 2 * C:3 * C]
                gms2m = gn_sb[:, 3 * C:4 * C]
                invc = gn_sb[:, 4 * C:4 * C + 1]
                epsc = gn_sb[:, 4 * C + 1:4 * C + 2]
                mean = smp.tile([8, C], f32, tag="mean")
                nc.vector.tensor_scalar(out=mean[:], in0=sall[:, 0:C],
                                        scalar1=invc, scalar2=None, op0=OP.mult)
                eh2 = smp.tile([8, C], f32, tag="eh2")
                nc.vector.tensor_scalar(out=eh2[:], in0=sall[:, C:2 * C],
                                        scalar1=invc, scalar2=None, op0=OP.mult)
                msq = smp.tile([8, C], f32, tag="msq")
                nc.vector.tensor_tensor(out=msq[:], in0=mean[:], in1=mean[:],
                                        op=OP.mult)
                var = smp.tile([8, C], f32, tag="var")
                nc.vector.tensor_tensor(out=msq[:], in0=msq[:], in1=gms2m,
                                        op=OP.mult)
                nc.vector.tensor_tensor(out=var[:], in0=eh2[:], in1=msq[:],
                                        op=OP.subtract)
                std = smp.tile([8, C], f32, tag="std")
                nc.scalar.activation(out=std[:], in_=var[:], func=AF.Sqrt,
                                     bias=epsc)
                ab = smp.tile([8, 2 * C], f32, tag="ab")
                nc.vector.reciprocal(std[:], std[:])
                nc.vector.tensor_tensor(out=ab[:, 0:C], in0=gw, in1=std[:],
                                        op=OP.mult)
                tm = smp.tile([8, C], f32, tag="tm")
                nc.vector.tensor_tensor(out=tm[:], in0=ab[:, 0:C], in1=mean[:],
                                        op=OP.mult)
                nc.vector.tensor_tensor(out=tm[:], in0=tm[:], in1=gms,
                                        op=OP.mult)
                nc.vector.tensor_tensor(out=ab[:, C:2 * C], in0=gb, in1=tm[:],
                                        op=OP.subtract)

                for g0 in range(0, TILES, G):
                    ng = min(G, TILES - g0)
                    obuf = ohp.tile([128, G * C], f32, tag="ob")
                    oht = ohp.tile([8, G * 128], f32, tag="oht")
                    nc.sync.dma_start(oht[:, 0:ng * 128],
                                      onehotT[:, g0 * 128:(g0 + ng) * 128])
                    for i in range(ng):
                        t = g0 + i
                        abpe = ps3.tile([128, 2 * C], f32, tag="abpe")
                        nc.tensor.matmul(abpe[:],
                                         lhsT=oht[:, i * 128:(i + 1) * 128],
                                         rhs=ab[:], start=True, stop=True)
                        nrm = smp.tile([128, C], f32, tag="nrm")
                        nc.vector.tensor_tensor(out=nrm[:],
                                                in0=h_sb[:, t * C:(t + 1) * C],
                                                in1=abpe[:, 0:C], op=OP.mult)
                        nc.vector.tensor_tensor(out=nrm[:], in0=nrm[:],
                                                in1=abpe[:, C:2 * C], op=OP.add)
                        nc.scalar.activation(out=obuf[:, i * C:(i + 1) * C],
                                             in_=nrm[:],
                                             func=AF.Gelu_apprx_tanh)
                    nfull = ng if g0 + ng < TILES else ng - 1
                    if nfull > 0:
                        nc.sync.dma_start(
                            out[g0 * 128:(g0 + nfull) * 128, :]
                            .rearrange("(g p) c -> p g c", p=128),
                            obuf[:, 0:nfull * C]
                            .rearrange("p (g c) -> p g c", c=C))
                    if g0 + ng == TILES:
                        nc.sync.dma_start(
                            out[(TILES - 1) * 128:(TILES - 1) * 128 + LAST_ROWS, :],
                            obuf[0:LAST_ROWS, (ng - 1) * C:ng * C])

    nc.compile()
    return nc


def kernel(**inputs):
    from concourse.bass_utils import run_bass_kernel_spmd

    cfg, in_maps = _host_prep(**inputs)
    nc = _build_nc(cfg)
    res = run_bass_kernel_spmd(nc, in_maps, core_ids=list(range(NCORE)))
    full = np.empty((N, C), _F32)
    for k in range(NCORE):
        full[cfg["own_all"][k]] = res.results[k]["out"]
    return full


# revision 34
# speedup vs baseline: 1.4872x; 1.0037x over previous
"""Multi-head GAT layer (4 heads, mean-aggregated) + residual + GraphNorm + gelu
on 8 Trainium2 NeuronCores (SPMD, one NEFF on all cores).

v3 strategy:
  - dst nodes are dealt to (core, tile, slot) round-robin by in-degree so per
    tile edge counts are balanced across cores (less SPMD max-padding).
  - NO on-device gather (SWDGE descriptor generation is serial on the Pool
    engine at ~2-8ns/descriptor -> ~1ms floor for per-edge gathers).  Instead
    the host lays out the per-edge source rows x[src_e] directly in transposed
    block-aligned order (xedgeT[f, edge_slot]) and the kernel streams them as
    bulk contiguous DMA.  Edge slots are grouped as 98 dst tiles x ~13 blocks
    of 128 edges.
  - Per 128-edge block, PE recomputes xl = x@W.T (and a_src = x@As.T); the
    one-hot-distributed a_dst (ape) accumulates onto a_src in one PSUM tile
    per dst tile, so leakyrelu+exp run batched per tile on ACT.
  - One-hot matrices (m1: [e,d], m2t: [d,e]) are uploaded as fp8 (exact 0/1)
    and used directly as matmul weights against bf16 moving operands.
  - The per-edge coefficient multiply (rhs = xl * ex) runs on two-block PSUM
    tiles and is split between DVE and ACT.
  - Softmax denominators ride as 4 extra columns of the agg matmul rhs.
  - GraphNorm stats accumulate in a persistent PSUM bank across all tiles,
    then AllReduce across the 8 cores.
"""
import numpy as np
import ml_dtypes

_BF16 = ml_dtypes.bfloat16
_FP8 = ml_dtypes.float8_e4m3
_F32 = np.float32

N, F, C, H, E, B = 100000, 128, 64, 4, 1200000, 8
NCORE = 8
NEG = 0.2
EPS = 1e-5
NOWN = N // NCORE             # 12500 dst nodes per core
TILES = (NOWN + 127) // 128   # 98 dst tiles per core (last has 84 slots)
LAST_ROWS = NOWN - (TILES - 1) * 128
G = 7                         # dst tiles per DMA group (98 = 14*7)
DVE_MOD = 8                   # duos with (idx % DVE_MOD) < DVE_CUT multiply on DVE
DVE_CUT = 7


def _cdiv(a, b):
    return (a + b - 1) // b


def _assign_nodes(deg):
    """Deal nodes (by descending degree) round-robin over the 784 (core,tile)
    pairs, honoring per-tile capacity. Returns [N] arrays core, tile, slot."""
    P = NCORE * TILES
    caps = np.full(P, 128, np.int64)
    caps[TILES - 1::TILES] = LAST_ROWS
    order = np.argsort(-deg, kind="stable")
    fill = np.zeros(P, np.int64)
    core = np.empty(N, np.int64)
    tile = np.empty(N, np.int64)
    slot = np.empty(N, np.int64)
    p = 0
    for n in order:
        while fill[p] >= caps[p]:
            p = (p + 1) % P
        core[n] = p // TILES
        tile[n] = p % TILES
        slot[n] = fill[p]
        fill[p] += 1
        p = (p + 1) % P
    return core, tile, slot


def _host_prep(x, edge_index, batch, W, att_src, att_dst, bias_gat, res_W,
               res_b, gn_weight, gn_bias, gn_mean_scale):
    x = np.asarray(x, _F32)
    W = np.asarray(W, _F32)
    att_src = np.asarray(att_src, _F32)
    att_dst = np.asarray(att_dst, _F32)
    res_W = np.asarray(res_W, _F32)
    batch = np.asarray(batch).astype(np.int64)

    # fused right matrix [F, 328] = [W.T | As.T | Ad.T | res_W.T]
    W3 = W.reshape(H, C, F)
    As = (att_src[:, :, None] * W3).sum(1)
    Ad = (att_dst[:, :, None] * W3).sum(1)
    Rcat = np.concatenate([W.T, As.T, Ad.T, res_W.T], axis=1).astype(_BF16)
    xT_bf = x.T.astype(_BF16)                        # [F, N]

    # ---- edges (+ self loops) ----
    loop = np.arange(N, dtype=np.int64)
    src = np.concatenate([np.asarray(edge_index[0]), loop]).astype(np.int64)
    dst = np.concatenate([np.asarray(edge_index[1]), loop]).astype(np.int64)
    deg = np.bincount(dst, minlength=N)
    n_core, n_tile, n_slot = _assign_nodes(deg)

    owner = n_core[dst]
    tl = n_tile[dst]
    dl = n_slot[dst]

    key = owner * TILES + tl
    counts = np.bincount(key, minlength=NCORE * TILES).reshape(NCORE, TILES)
    K_t = counts.max(axis=0).astype(np.int64)        # [TILES]
    nb_t = _cdiv(K_t, 128)
    K_pad = nb_t * 128

    order = np.lexsort((tl, owner))
    s_src, s_dl = src[order], dl[order]
    gstart = np.zeros(NCORE * TILES + 1, np.int64)
    gstart[1:] = np.cumsum(counts.flatten())

    # ---- static block bookkeeping (same on all cores) ----
    groups = [list(range(g0, min(g0 + G, TILES))) for g0 in range(0, TILES, G)]
    TOTBLK = int(nb_t.sum())
    tile_blocks = [[] for _ in range(TILES)]   # (global block idx, group col)
    seg_meta = []                              # per tile: (t, blk0)
    blk = 0
    group_B0 = []
    for gtiles in groups:
        group_B0.append(blk)
        xoff = 0
        for t in gtiles:
            nb = int(nb_t[t])
            seg_meta.append((t, blk))
            for k in range(nb):
                tile_blocks[t].append((blk, xoff + k * 128))
                blk += 1
            xoff += nb * 128
    assert blk == TOTBLK
    gb_per_group = [int(nb_t[np.array(g)].sum()) for g in groups]
    MAXGB = max(gb_per_group)
    MAXNST = max(len(tb) for tb in tile_blocks)

    own_all = []
    for k in range(NCORE):
        own = np.where(n_core == k)[0]
        own = own[np.argsort((n_tile[own] * 128 + n_slot[own]), kind="stable")]
        own_all.append(own)

    # ---- per-core tensors ----
    in_maps = []
    for k in range(NCORE):
        xedgeT = np.zeros((F, TOTBLK * 128), _BF16)
        dlflat = np.full(TOTBLK * 128, -1, np.int64)
        for (t, blk0) in seg_meta:
            gi = k * TILES + t
            n = int(counts[k, t])
            if n == 0:
                continue
            a = int(gstart[gi])
            sl = slice(blk0 * 128, blk0 * 128 + n)
            xedgeT[:, sl] = xT_bf[:, s_src[a:a + n]]
            dlflat[sl] = s_dl[a:a + n]
        dlb = dlflat.reshape(TOTBLK, 128)
        m1arr = (dlb[:, :, None] == np.arange(128)[None, None, :])
        m1_all = np.ascontiguousarray(
            m1arr.transpose(1, 0, 2).reshape(128, TOTBLK * 128)).astype(_FP8)
        m2t_all = np.ascontiguousarray(
            m1arr.transpose(2, 0, 1).reshape(128, TOTBLK * 128)).astype(_FP8)

        own = own_all[k]
        rowpos = n_tile[own] * 128 + n_slot[own]
        xTo = np.zeros((F, TILES * 128), _BF16)
        xTo[:, rowpos] = xT_bf[:, own]
        bown = batch[own]
        onehot_b = np.zeros((128, TILES * 8), _BF16)
        onehot_b[rowpos % 128, (rowpos // 128) * 8 + bown] = 1.0
        onehotT = np.zeros((8, TILES * 128), _F32)
        onehotT[bown, rowpos] = 1.0

        in_maps.append({
            "xedgeT": xedgeT, "Rcat": Rcat,
            "m1_all": m1_all, "m2t_all": m2t_all,
            "xTo": xTo, "onehot_b": onehot_b, "onehotT": onehotT,
        })

    bc_row = np.tile((np.asarray(bias_gat, _F32)
                      + np.asarray(res_b, _F32))[None, :], (128, 1))
    alpha_t = np.full((128, 1), NEG, _F32)
    gms = np.asarray(gn_mean_scale, _F32)
    cnt = np.bincount(batch, minlength=B).astype(_F32)
    gn_pack = np.zeros((8, 4 * C + 2), _F32)
    gn_pack[:, 0:C] = np.asarray(gn_weight, _F32)[None, :]
    gn_pack[:, C:2 * C] = np.asarray(gn_bias, _F32)[None, :]
    gn_pack[:, 2 * C:3 * C] = gms[None, :]
    gn_pack[:, 3 * C:4 * C] = (gms * (2.0 - gms))[None, :]
    gn_pack[:, 4 * C] = 1.0 / cnt
    gn_pack[:, 4 * C + 1] = EPS
    for m in in_maps:
        m.update({"bc_row": bc_row, "alpha_t": alpha_t, "gn_pack": gn_pack})

    cfg = {
        "groups": groups, "group_B0": group_B0, "gb_per_group": gb_per_group,
        "tile_blocks": tile_blocks, "TOTBLK": TOTBLK,
        "MAXGB": MAXGB, "MAXNST": MAXNST, "own_all": own_all, "nb_t": nb_t,
    }
    return cfg, in_maps


def _build_nc(cfg, debug=False):
    import concourse.bacc as bacc
    import concourse.mybir as mybir
    import concourse.tile as tile

    AF = mybir.ActivationFunctionType
    OP = mybir.AluOpType
    f32 = mybir.dt.float32
    bf16 = mybir.dt.bfloat16
    fp8 = mybir.dt.float8e4

    groups = cfg["groups"]
    group_B0 = cfg["group_B0"]
    gb_per_group = cfg["gb_per_group"]
    tile_blocks = cfg["tile_blocks"]
    TOTBLK = cfg["TOTBLK"]
    MAXGB, MAXNST = cfg["MAXGB"], cfg["MAXNST"]

    nc = bacc.Bacc("TRN2", target_bir_lowering=False)

    xedgeT = nc.declare_dram_parameter("xedgeT", [F, TOTBLK * 128], bf16, isOutput=False)
    Rcat = nc.declare_dram_parameter("Rcat", [F, 328], bf16, isOutput=False)
    m1_all = nc.declare_dram_parameter("m1_all", [128, TOTBLK * 128], fp8, isOutput=False)
    m2t_all = nc.declare_dram_parameter("m2t_all", [128, TOTBLK * 128], fp8, isOutput=False)
    xTo = nc.declare_dram_parameter("xTo", [F, TILES * 128], bf16, isOutput=False)
    onehot_b = nc.declare_dram_parameter("onehot_b", [128, TILES * 8], bf16, isOutput=False)
    onehotT = nc.declare_dram_parameter("onehotT", [8, TILES * 128], f32, isOutput=False)
    bc_row = nc.declare_dram_parameter("bc_row", [128, C], f32, isOutput=False)
    alpha_t = nc.declare_dram_parameter("alpha_t", [128, 1], f32, isOutput=False)
    gn_pack = nc.declare_dram_parameter("gn_pack", [8, 4 * C + 2], f32, isOutput=False)
    out = nc.declare_dram_parameter("out", [NOWN, C], f32, isOutput=True)

    cc_in = nc.dram_tensor("cc_in", [8, 2 * C], f32)
    cc_out = nc.dram_tensor("cc_out", [8, 2 * C], f32)
    if debug:
        dbg_h = nc.declare_dram_parameter("dbg_h", [128, TILES * C], f32, isOutput=True)
        dbg_adst = nc.declare_dram_parameter("dbg_adst", [128, TILES * 4], f32, isOutput=True)
        dbg_resid = nc.declare_dram_parameter("dbg_resid", [128, TILES * C], f32, isOutput=True)
        dbg_stats = nc.declare_dram_parameter("dbg_stats", [8, 2 * C], f32, isOutput=True)
        dbg_ex = nc.declare_dram_parameter("dbg_ex", [128, MAXNST * 4], f32, isOutput=True)

    with tile.TileContext(nc) as tc:
        with (
            tc.tile_pool(name="const", bufs=1) as cp,
            tc.tile_pool(name="persist", bufs=1) as pers,
            tc.tile_pool(name="xload", bufs=2) as xp,
            tc.tile_pool(name="xe", bufs=2) as xep,
            tc.tile_pool(name="m1pool", bufs=2) as mp1,
            tc.tile_pool(name="m2pool", bufs=2) as mp2,
            tc.tile_pool(name="rhsp", bufs=3) as rhp,
            tc.tile_pool(name="small", bufs=6) as smp,
        ):
            # ---- constants ----
            rc_sb = cp.tile([F, 328], bf16)
            nc.sync.dma_start(rc_sb[:], Rcat[:])
            bc_sb = cp.tile([128, C], f32)
            nc.sync.dma_start(bc_sb[:], bc_row[:])
            ohb_sb = cp.tile([128, TILES * 8], bf16)
            nc.sync.dma_start(ohb_sb[:], onehot_b[:])
            al_sb = cp.tile([128, 1], f32)
            nc.sync.dma_start(al_sb[:], alpha_t[:])
            gn_sb = cp.tile([8, 4 * C + 2], f32)
            nc.sync.dma_start(gn_sb[:], gn_pack[:])


            adst_sb = pers.tile([128, TILES * 4], f32)
            resid_sb = pers.tile([128, TILES * C], bf16)
            h_sb = pers.tile([128, TILES * C], bf16)

            # ---- phases 1+2 interleaved per group ----
            with (
                tc.tile_pool(name="psum_xl", bufs=3, space="PSUM") as pxl,
                tc.tile_pool(name="psum_lr", bufs=2, space="PSUM") as plr,
                tc.tile_pool(name="psum_agg", bufs=2, space="PSUM") as pag,
                tc.tile_pool(name="psum_stat", bufs=1, space="PSUM") as pst,
            ):
                stats_ps = pst.tile([8, 2 * C], f32)
                duo_ctr = [0]
                for gi, gtiles in enumerate(groups):
                    B0 = group_B0[gi]
                    GB = gb_per_group[gi]
                    # phase 1 part: owned-node sweep -> a_dst + residual
                    g0 = gtiles[0]
                    ng = len(gtiles)
                    xs = xp.tile([F, G * 128], bf16, tag="xo")
                    nc.sync.dma_start(xs[:, 0:ng * 128],
                                      xTo[:, g0 * 128:(g0 + ng) * 128])
                    for i in range(ng):
                        t = g0 + i
                        ps = pxl.tile([128, 512], f32, tag="xlps")
                        nc.tensor.matmul(ps[:, 0:68],
                                         lhsT=xs[:, i * 128:(i + 1) * 128],
                                         rhs=rc_sb[:, 260:328],
                                         start=True, stop=True,
                                         skip_group_check=True)
                        nc.vector.tensor_copy(adst_sb[:, t * 4:(t + 1) * 4],
                                              ps[:, 0:4])
                        nc.vector.tensor_tensor(
                            out=resid_sb[:, t * C:(t + 1) * C],
                            in0=ps[:, 4:68], in1=bc_sb[:], op=OP.add)
                    # phase 2 part: edge sweep
                    xe = xep.tile([F, MAXGB * 128], bf16, tag="xe")
                    nc.sync.dma_start(xe[:, 0:GB * 128],
                                      xedgeT[:, B0 * 128:(B0 + GB) * 128])
                    m1s = mp1.tile([128, MAXGB * 128], fp8, tag="m1")
                    nc.sync.dma_start(m1s[:, 0:GB * 128],
                                      m1_all[:, B0 * 128:(B0 + GB) * 128])
                    m2s = mp2.tile([128, MAXGB * 128], fp8, tag="m2")
                    nc.sync.dma_start(m2s[:, 0:GB * 128],
                                      m2t_all[:, B0 * 128:(B0 + GB) * 128])

                    for t in gtiles:
                        blocks = tile_blocks[t]
                        nst = len(blocks)
                        adstb = smp.tile([128, 4], bf16, tag="adstb")
                        nc.vector.tensor_copy(adstb[:],
                                              adst_sb[:, t * 4:(t + 1) * 4])
                        # a_src + distributed a_dst -> lr (one psum tile)
                        ps_lr = plr.tile([128, MAXNST * 4], f32, tag="lr")
                        for bj, (Bg, gcol) in enumerate(blocks):
                            xel = xe[:, gcol:gcol + 128]
                            nc.tensor.matmul(
                                ps_lr[:, bj * 4:(bj + 1) * 4], lhsT=xel,
                                rhs=rc_sb[:, 256:260], start=True, stop=False,
                                skip_group_check=True)
                            mb = (Bg - B0) * 128
                            nc.tensor.matmul(
                                ps_lr[:, bj * 4:(bj + 1) * 4],
                                lhsT=m2s[:, mb:mb + 128],
                                rhs=adstb[:], start=False, stop=True,
                                skip_group_check=True)
                        ex32 = smp.tile([128, MAXNST * 4], f32, tag="ex32")
                        nc.scalar.activation(out=ex32[:, 0:nst * 4],
                                             in_=ps_lr[:, 0:nst * 4],
                                             func=AF.Prelu, alpha=al_sb[:, 0:1])
                        nc.scalar.activation(out=ex32[:, 0:nst * 4],
                                             in_=ex32[:, 0:nst * 4], func=AF.Exp)
                        if debug and t == 0:
                            nc.sync.dma_start(dbg_ex[:, 0:nst * 4],
                                              ex32[:, 0:nst * 4])
                        rhs_t = rhp.tile([128, MAXNST * 260], bf16, tag="rhs")
                        nc.vector.tensor_copy(
                            rhs_t[:, 0:nst * 260]
                            .rearrange("p (j x) -> p j x", x=260)[:, :, 256:260],
                            ex32[:, 0:nst * 4]
                            .rearrange("p (j h) -> p j h", h=4))
                        agg = pag.tile([128, 260], f32, tag="agg")
                        for d0 in range(0, nst, 2):
                            nd = min(2, nst - d0)
                            ps_xl = pxl.tile([128, 512], f32, tag="xlps")
                            for u in range(nd):
                                bj = d0 + u
                                Bg, gcol = blocks[bj]
                                nc.tensor.matmul(
                                    ps_xl[:, u * 256:(u + 1) * 256],
                                    lhsT=xe[:, gcol:gcol + 128],
                                    rhs=rc_sb[:, 0:256],
                                    start=True, stop=True,
                                    skip_group_check=True)
                            if duo_ctr[0] % DVE_MOD < DVE_CUT:
                                nc.vector.tensor_tensor(
                                    out=rhs_t[:, d0 * 260:(d0 + nd) * 260]
                                    .rearrange("p (j x) -> p j x", x=260)
                                    [:, :, 0:256]
                                    .rearrange("p j (h c) -> p j h c", h=H),
                                    in0=ps_xl[:, 0:nd * 256]
                                    .rearrange("p (j h c) -> p j h c", h=H, c=C),
                                    in1=ex32[:, d0 * 4:(d0 + nd) * 4]
                                    .rearrange("p (j h) -> p j h", h=4)
                                    .to_broadcast([128, nd, H, C]),
                                    op=OP.mult)
                            else:
                                for u in range(nd):
                                    bj = d0 + u
                                    for h in range(H):
                                        nc.scalar.activation(
                                            out=rhs_t[:, bj * 260 + h * C:
                                                      bj * 260 + (h + 1) * C],
                                            in_=ps_xl[:, u * 256 + h * C:
                                                      u * 256 + (h + 1) * C],
                                            func=AF.Copy,
                                            scale=ex32[:, bj * 4 + h:
                                                       bj * 4 + h + 1])
                            duo_ctr[0] += 1
                            for u in range(nd):
                                bj = d0 + u
                                Bg, gcol = blocks[bj]
                                mb = (Bg - B0) * 128
                                nc.tensor.matmul(
                                    agg[:], lhsT=m1s[:, mb:mb + 128],
                                    rhs=rhs_t[:, bj * 260:(bj + 1) * 260],
                                    start=(bj == 0), stop=(bj == nst - 1))
                        # combine heads, add residual
                        dn = smp.tile([128, 4], f32, tag="dn")
                        nc.vector.tensor_scalar(
                            out=dn[:], in0=agg[:, 256:260], scalar1=1e-6,
                            scalar2=None, op0=OP.add)
                        recip = smp.tile([128, 4], f32, tag="recip")
                        nc.vector.reciprocal(recip[:], dn[:])
                        hacc = smp.tile([128, C], f32, tag="hacc")
                        nc.vector.tensor_scalar(
                            out=hacc[:], in0=agg[:, 0:C], scalar1=recip[:, 0:1],
                            scalar2=None, op0=OP.mult)
                        for h in range(1, H):
                            nc.vector.scalar_tensor_tensor(
                                out=hacc[:], in0=agg[:, h * C:(h + 1) * C],
                                scalar=recip[:, h:h + 1], in1=hacc[:],
                                op0=OP.mult, op1=OP.add)
                        hsl = h_sb[:, t * C:(t + 1) * C]
                        nc.vector.scalar_tensor_tensor(
                            out=hsl, in0=hacc[:], scalar=1.0 / H,
                            in1=resid_sb[:, t * C:(t + 1) * C],
                            op0=OP.mult, op1=OP.add)
                        # graphnorm partial stats (accumulate in psum)
                        sq = smp.tile([128, C], bf16, tag="sq")
                        nc.scalar.square(sq[:], hsl)
                        nc.tensor.matmul(stats_ps[:, 0:C],
                                         lhsT=ohb_sb[:, t * 8:(t + 1) * 8],
                                         rhs=hsl, start=(t == 0),
                                         stop=(t == TILES - 1),
                                         skip_group_check=True)
                        nc.tensor.matmul(stats_ps[:, C:2 * C],
                                         lhsT=ohb_sb[:, t * 8:(t + 1) * 8],
                                         rhs=sq[:], start=(t == 0),
                                         stop=(t == TILES - 1),
                                         skip_group_check=True)
                stats_sb = pers.tile([8, 2 * C], f32)
                nc.vector.tensor_copy(stats_sb[:], stats_ps[:])
            if debug:
                nc.sync.dma_start(dbg_adst[:], adst_sb[:])
                nc.sync.dma_start(dbg_stats[:], stats_sb[:])
                nc.gpsimd.dma_start(dbg_resid[:], resid_sb[:])
                nc.gpsimd.dma_start(dbg_h[:], h_sb[:])

            # ---- phase 3: AllReduce stats, normalize, gelu, write out ----
            with tc.tile_pool(name="psum3", bufs=2, space="PSUM") as ps3, \
                 tc.tile_pool(name="ohtp", bufs=2) as ohp:
                nc.gpsimd.dma_start(cc_in[:], stats_sb[:])
                nc.gpsimd.collective_compute(
                    "AllReduce", OP.add,
                    replica_groups=[list(range(NCORE))],
                    ins=[cc_in[:]], outs=[cc_out[:]])
                sall = smp.tile([8, 2 * C], f32, tag="sall")
                nc.sync.dma_start(sall[:], cc_out[:])
                gw = gn_sb[:, 0:C]
                gb = gn_sb[:, C:2 * C]
                gms = gn_sb[:, 2 * C:3 * C]
                gms2m = gn_sb[:, 3 * C:4 * C]
                invc = gn_sb[:, 4 * C:4 * C + 1]
                epsc = gn_sb[:, 4 * C + 1:4 * C + 2]
                mean = smp.tile([8, C], f32, tag="mean")
                nc.vector.tensor_scalar(out=mean[:], in0=sall[:, 0:C],
                                        scalar1=invc, scalar2=None, op0=OP.mult)
                eh2 = smp.tile([8, C], f32, tag="eh2")
                nc.vector.tensor_scalar(out=eh2[:], in0=sall[:, C:2 * C],
                                        scalar1=invc, scalar2=None, op0=OP.mult)
                msq = smp.tile([8, C], f32, tag="msq")
                nc.vector.tensor_tensor(out=msq[:], in0=mean[:], in1=mean[:],
                                        op=OP.mult)
                var = smp.tile([8, C], f32, tag="var")
                nc.vector.tensor_tensor(out=msq[:], in0=msq[:], in1=gms2m,
                                        op=OP.mult)
                nc.vector.tensor_tensor(out=var[:], in0=eh2[:], in1=msq[:],
                                        op=OP.subtract)
                std = smp.tile([8, C], f32, tag="std")
                nc.scalar.activation(out=std[:], in_=var[:], func=AF.Sqrt,
                                     bias=epsc)
                ab = smp.tile([8, 2 * C], f32, tag="ab")
                nc.vector.reciprocal(std[:], std[:])
                nc.vector.tensor_tensor(out=ab[:, 0:C], in0=gw, in1=std[:],
                                        op=OP.mult)
                tm = smp.tile([8, C], f32, tag="tm")
                nc.vector.tensor_tensor(out=tm[:], in0=ab[:, 0:C], in1=mean[:],
                                        op=OP.mult)
                nc.vector.tensor_tensor(out=tm[:], in0=tm[:], in1=gms,
                                        op=OP.mult)
                nc.vector.tensor_tensor(out=ab[:, C:2 * C], in0=gb, in1=tm[:],
                                        op=OP.subtract)

                for g0 in range(0, TILES, G):
                    ng = min(G, TILES - g0)
                    obuf = ohp.tile([128, G * C], f32, tag="ob")
                    oht = ohp.tile([8, G * 128], f32, tag="oht")
                    nc.sync.dma_start(oht[:, 0:ng * 128],
                                      onehotT[:, g0 * 128:(g0 + ng) * 128])
                    for i0 in range(0, ng, 2):
                        ni = min(2, ng - i0)
                        t0 = g0 + i0
                        # a/b rows for 2 tiles in one psum tile: [a0|a1|b0|b1]
                        abpe = ps3.tile([128, 4 * C], f32, tag="abpe")
                        for u in range(ni):
                            nc.tensor.matmul(
                                abpe[:, u * 2 * C:(u + 1) * 2 * C],
                                lhsT=oht[:, (i0 + u) * 128:(i0 + u + 1) * 128],
                                rhs=ab[:], start=True, stop=True,
                                skip_group_check=True)
                        nrm = smp.tile([128, 2 * C], f32, tag="nrm")
                        abv = abpe[:].rearrange("p (u x c) -> p u x c", u=2, c=C)
                        nc.vector.tensor_tensor(
                            out=nrm[:, 0:ni * C]
                            .rearrange("p (u c) -> p u c", c=C),
                            in0=h_sb[:, t0 * C:(t0 + ni) * C]
                            .rearrange("p (u c) -> p u c", c=C),
                            in1=abv[:, 0:ni, 0, :], op=OP.mult)
                        nc.vector.tensor_tensor(
                            out=nrm[:, 0:ni * C]
                            .rearrange("p (u c) -> p u c", c=C),
                            in0=nrm[:, 0:ni * C]
                            .rearrange("p (u c) -> p u c", c=C),
                            in1=abv[:, 0:ni, 1, :], op=OP.add)
                        nc.scalar.activation(out=obuf[:, i0 * C:(i0 + ni) * C],
                                             in_=nrm[:, 0:ni * C],
                                             func=AF.Gelu_apprx_tanh)
                    nfull = ng if g0 + ng < TILES else ng - 1
                    if nfull > 0:
                        nc.sync.dma_start(
                            out[g0 * 128:(g0 + nfull) * 128, :]
                            .rearrange("(g p) c -> p g c", p=128),
                            obuf[:, 0:nfull * C]
                            .rearrange("p (g c) -> p g c", c=C))
                    if g0 + ng == TILES:
                        nc.sync.dma_start(
                            out[(TILES - 1) * 128:(TILES - 1) * 128 + LAST_ROWS, :],
                            obuf[0:LAST_ROWS, (ng - 1) * C:ng * C])

    nc.compile()
    return nc


def kernel(**inputs):
    from concourse.bass_utils import run_bass_kernel_spmd

    cfg, in_maps = _host_prep(**inputs)
    nc = _build_nc(cfg)
    res = run_bass_kernel_spmd(nc, in_maps, core_ids=list(range(NCORE)))
    full = np.empty((N, C), _F32)
    for k in range(NCORE):
        full[cfg["own_all"][k]] = res.results[k]["out"]
    return full
